# revision 1
# baseline (speedup 1.0000x reference)
"""Causal self-attention v2 (B=4, T=2048, C=1024, H=16) on 8 trn2 NeuronCores.

Sharding: core = (batch b, head-group hg), b = core//2, hg = core%2 (Megatron
column-parallel qkv / row-parallel proj); host sums the two partial outputs.

v2 changes vs baseline:
  - attention operands in bf16 (qt/kt/pt/v/yt/wp): same PE throughput, but
    allows trimming matmuls below 256-wide moving (fp32r would drop to 1/4
    rate), halves SBUF, enables DVE 2x mask ops.
  - phase C group = one 128-kv chunk x both heads: 2 S matmuls -> one
    [128,1024] exp -> (diagonal only) 128x128 triangle mask mul -> 2 AV
    matmuls.  Dead (above-diagonal) columns are never computed: S, exp and
    AV are trimmed to the live column range; no gpsimd memsets.
  - qb-major unit order so phase D per q-block can interleave later.
  - phase D: one 1024-wide bf16 matmul per (tt, pair).
"""

import functools

import numpy as np

B, T, C, H = 4, 2048, 1024, 16
HD = C // H  # 64
N_CORES = 8
HG = 2  # head groups
NH = H // HG  # heads per core = 8
NP = NH // 2  # head pairs per core = 4
TT = T // 128  # 16 t-tiles
TB = T // 512  # 4 t-blocks
CK = C // 128  # 8 c-chunks


def _build(rep=1, la=2, sbufs=3, ybufs=2, ptbufs=6, ablate="full"):
    import concourse.bass as bass
    import concourse.mybir as mybir
    import concourse.tile as tile
    from concourse import bacc

    f32 = mybir.dt.float32
    f32r = mybir.dt.float32r
    bf16 = mybir.dt.bfloat16

    nc = bacc.Bacc("TRN2", target_bir_lowering=False, debug=False)

    xt_d = nc.dram_tensor("xt", [C, T], bf16, kind="ExternalInput")
    wq_d = nc.dram_tensor("wq", [C, 512], bf16, kind="ExternalInput")
    wk_d = nc.dram_tensor("wk", [C, 512], bf16, kind="ExternalInput")
    wv_d = nc.dram_tensor("wv", [C, 512], bf16, kind="ExternalInput")
    wp_d = nc.dram_tensor("wp", [512, C], bf16, kind="ExternalInput")
    tri_d = nc.dram_tensor("tri", [128, 128], bf16, kind="ExternalInput")
    out_d = nc.dram_tensor("out", [T, C], f32, kind="ExternalOutput")

    do_attn = ablate in ("full", "noav")
    do_av = ablate in ("full",)
    do_d = ablate in ("full", "noattn")

    with tile.TileContext(nc) as tc:
        with tc.tile_pool(name="persist", bufs=1) as persist:
            qt_sb = persist.tile([128, NP, T], bf16, tag="qt")
            kt_sb = persist.tile([128, NP, T], bf16, tag="kt")
            ones_sb = persist.tile([128, 64], bf16, tag="ones")
            nc.vector.memset(ones_sb[:], 1.0)

            def body():
                # strictly nested (LIFO) pool lifetimes
                vp_cm = tc.tile_pool(name="vp", bufs=1)
                xtp_cm = tc.tile_pool(name="xtp", bufs=1)
                wqk_cm = tc.tile_pool(name="wqk", bufs=1)
                vp = vp_cm.__enter__()
                xtp = xtp_cm.__enter__()
                wqk = wqk_cm.__enter__()

                v_sb = vp.tile([128, TT, NH, HD + 1], bf16, tag="v")
                # ones columns of V': memset whole tile; V writes overlay 0:HD
                nc.vector.memset(v_sb[:], 1.0)

                wq_sb = wqk.tile([128, CK, 512], bf16, tag="wq")
                wk_sb = wqk.tile([128, CK, 512], bf16, tag="wk")
                wv_sb = wqk.tile([128, CK, 512], bf16, tag="wv")
                wp_sb = wqk.tile([128, NP, C], bf16, tag="wp")
                tri_sb = wqk.tile([128, 128], bf16, tag="tri")
                xt_sb = xtp.tile([128, CK, T], bf16, tag="xt")
                # exp-table preload on the idle ACT engine during the DMA head
                warm = wqk.tile([128, 16], bf16, tag="warm")
                nc.scalar.activation(
                    warm[0:1, :], ones_sb[0:1, 0:16],
                    mybir.ActivationFunctionType.Exp, scale=0.125,
                )
                # chunked DMAs in consumption order; tri first (first-unit masks),
                # wv interleaved so the early V-projection tasks aren't starved
                nc.sync.dma_start(tri_sb[:], tri_d[:, :])
                for k in range(CK):
                    ksl = slice(k * 128, (k + 1) * 128)
                    nc.sync.dma_start(wq_sb[:, k, :], wq_d[ksl, :])
                    nc.sync.dma_start(wk_sb[:, k, :], wk_d[ksl, :])
                    nc.sync.dma_start(xt_sb[:, k, :], xt_d[ksl, :])
                    nc.sync.dma_start(wv_sb[:, k, :], wv_d[ksl, :])
                nc.sync.dma_start(wp_sb[:], wp_d.rearrange("(a p) n -> p a n", p=128))

                # ---- phases C+D pools (opened after xt is freed)
                with (
                    tc.tile_pool(name="persist2", bufs=1) as persist2,
                    tc.tile_pool(name="ptp", bufs=ptbufs) as ptp,
                    tc.tile_pool(name="recp", bufs=2) as recp,
                    tc.tile_pool(name="outp", bufs=3) as outp,
                ):
                    yt_sb = persist2.tile([128, NP, T], bf16, tag="yt")
                    if ablate in ("noattn",):
                        nc.vector.memset(yt_sb[:], 0.001)
                    with (
                        tc.tile_pool(name="pss", bufs=sbufs, space="PSUM") as pss,
                        tc.tile_pool(name="psy", bufs=ybufs, space="PSUM") as psy,
                    ):
                        # ---- phase C: qb-major, group = 1 kv chunk x 2 heads
                        tasks = []
                        if do_attn:
                            for tb in range(TB):
                                tasks.append(("a", 0, tb, 0, 0))
                            for tt in range(4):
                                tasks.append(("b", tt, 0, 0, 0))
                            for p in range(NP):
                                for qb in range(TB):
                                    ng = 4 * (qb + 1)
                                    for g in range(ng):
                                        tasks.append(("g", qb, p, g, ng))
                                    if p == 0 and qb < 3:
                                        for tt in range(4 * qb + 4, 4 * qb + 8):
                                            tasks.append(("b", tt, 0, 0, 0))
                                    if qb == 0 and p < NP - 1:
                                        for tb in range(TB):
                                            tasks.append(("a", p + 1, tb, 0, 0))
                            if do_d:
                                for tt in range(TT):
                                    tasks.append(("d", tt, 0, 0, 0))

                        pt_store = {}
                        y_store = {}

                        def emit_front(idx):
                            kind, qb, p, g, ng = tasks[idx]
                            if kind != "g":
                                return
                            r = g - 4 * qb  # >=0: diagonal-straddling chunk
                            lo = 128 * r if r > 0 else 0
                            ksl = slice(g * 128, (g + 1) * 128)
                            s = pss.tile([128, 1024], f32, tag="s", name=f"s_{idx}")
                            for hf in range(2):
                                nc.tensor.matmul(
                                    s[:, 512 * hf + lo : 512 * (hf + 1)],
                                    kt_sb[64 * hf : 64 * (hf + 1), p, ksl],
                                    qt_sb[64 * hf : 64 * (hf + 1), p,
                                          qb * 512 + lo : (qb + 1) * 512],
                                    start=True, stop=True,
                                )
                            pt = ptp.tile([128, 1024], bf16, tag="pt", name=f"pt_{idx}")
                            if r <= 0:
                                nc.scalar.activation(
                                    pt[:], s[:], mybir.ActivationFunctionType.Exp,
                                    scale=0.125,
                                )
                            else:
                                for hf in range(2):
                                    fsl = slice(512 * hf + lo, 512 * (hf + 1))
                                    nc.scalar.activation(
                                        pt[:, fsl], s[:, fsl],
                                        mybir.ActivationFunctionType.Exp,
                                        scale=0.125,
                                    )
                            if r >= 0:
                                # triangle mask on the diagonal 128-wide strip
                                for hf in range(2):
                                    dsl = slice(512 * hf + lo, 512 * hf + lo + 128)
                                    nc.vector.tensor_mul(pt[:, dsl], pt[:, dsl], tri_sb[:])
                            pt_store[idx] = pt

                        def emit_av(idx):
                            kind, qb, p, g, ng = tasks[idx]
                            if kind == "a":
                                pp, tb = qb, p
                                psqk = pss.tile([128, 1024], f32, tag="s", name=f"psqk{pp}_{tb}")
                                for k in range(CK):
                                    st, sp = k == 0, k == CK - 1
                                    nc.tensor.matmul(
                                        psqk[:, 0:512],
                                        wq_sb[:, k, pp * 128 : (pp + 1) * 128],
                                        xt_sb[:, k, tb * 512 : (tb + 1) * 512],
                                        start=st, stop=sp,
                                    )
                                    nc.tensor.matmul(
                                        psqk[:, 512:1024],
                                        wk_sb[:, k, pp * 128 : (pp + 1) * 128],
                                        xt_sb[:, k, tb * 512 : (tb + 1) * 512],
                                        start=st, stop=sp,
                                    )
                                tsl = slice(tb * 512, (tb + 1) * 512)
                                nc.vector.tensor_copy(qt_sb[:, pp, tsl], psqk[:, 0:512])
                                nc.vector.tensor_copy(kt_sb[:, pp, tsl], psqk[:, 512:1024])
                                return
                            if kind == "b":
                                tt = qb
                                psv = pss.tile([128, 512], f32, tag="s", name=f"psv{tt}")
                                for k in range(CK):
                                    nc.tensor.matmul(
                                        psv[:],
                                        xt_sb[:, k, tt * 128 : (tt + 1) * 128],
                                        wv_sb[:, k, :],
                                        start=(k == 0), stop=(k == CK - 1),
                                    )
                                nc.vector.tensor_copy(
                                    v_sb[:, tt, :, 0:HD],
                                    psv[:].rearrange("p (h e) -> p h e", e=HD),
                                )
                                return
                            if kind == "d":
                                tt = qb
                                po = pss.tile([128, 1024], f32, tag="s", name=f"po{tt}")
                                for pp in range(NP):
                                    for nb in range(2):
                                        nc.tensor.matmul(
                                            po[:, nb * 512 : (nb + 1) * 512],
                                            yt_sb[:, pp, tt * 128 : (tt + 1) * 128],
                                            wp_sb[:, pp, nb * 512 : (nb + 1) * 512],
                                            start=(pp == 0), stop=(pp == NP - 1),
                                        )
                                ot = outp.tile([128, 1024], f32, tag="ot")
                                nc.vector.tensor_copy(ot[:], po[:])
                                nc.sync.dma_start(
                                    out_d[tt * 128 : (tt + 1) * 128, :], ot[:],
                                )
                                return
                            r = g - 4 * qb
                            lo = 128 * r if r > 0 else 0
                            pt = pt_store.pop(idx)
                            if (qb, p) not in y_store:
                                ya = psy.tile([65, 512], f32, tag="y", name=f"ya_{qb}_{p}")
                                yb = psy.tile([65, 512], f32, tag="y", name=f"yb_{qb}_{p}")
                                y_store[(qb, p)] = (ya, yb)
                            ya, yb = y_store[(qb, p)]
                            if do_av:
                                for hf, yy in ((0, ya), (1, yb)):
                                    nc.tensor.matmul(
                                        yy[:, lo:512],
                                        v_sb[:, g, 2 * p + hf, :],
                                        pt[:, 512 * hf + lo : 512 * (hf + 1)],
                                        start=(g == 0), stop=(g == ng - 1),
                                        skip_group_check=True,
                                    )
                            if g != ng - 1:
                                return
                            if not do_av:
                                nc.vector.memset(ya[:], 1.0)
                                nc.vector.memset(yb[:], 1.0)
                            # normalize: yt = y[0:64] * (1 / rowsum) broadcast
                            qsl = slice(qb * 512, (qb + 1) * 512)
                            for hi, yy in ((0, ya), (1, yb)):
                                rs = recp.tile([128, 512], f32, tag="rs", name=f"rs_{qb}_{p}_{hi}")
                                nc.vector.tensor_copy(rs[0:1, :], yy[64:65, :])
                                rec = recp.tile([128, 512], f32, tag="rec", name=f"rec_{qb}_{p}_{hi}")
                                nc.vector.reciprocal_approx_fast(rec[0:1, :], rs[0:1, :])
                                recb = recp.tile([128, 512], bf16, tag="recb", name=f"recb_{qb}_{p}_{hi}")
                                nc.vector.tensor_copy(recb[0:1, :], rec[0:1, :])
                                bc = pss.tile([64, 512], f32, tag="s", name=f"bc_{qb}_{p}_{hi}")
                                nc.tensor.matmul(
                                    bc[:], ones_sb[0:1, 0:64], recb[0:1, :],
                                    start=True, stop=True,
                                )
                                bc_sb = recp.tile([64, 512], f32, tag="bc", name=f"bcs_{qb}_{p}_{hi}")
                                nc.vector.tensor_copy(bc_sb[:], bc[:])
                                nc.vector.tensor_mul(
                                    yt_sb[hi * 64 : (hi + 1) * 64, p, qsl],
                                    yy[0:64, :], bc_sb[:],
                                )
                            del y_store[(qb, p)]

                        n = len(tasks)
                        for j in range(min(la, n)):
                            emit_front(j)
                        for i in range(n):
                            if i + la < n:
                                emit_front(i + la)
                            emit_av(i)

                    # ---- phase D fallback (only when attention is ablated).
                    with (
                        tc.tile_pool(name="pso", bufs=2, space="PSUM") as pso,
                        tc.tile_pool(name="outp2", bufs=3) as outp2,
                    ):
                        for tt in (range(TT) if (do_d and not do_attn) else range(0)):
                            po = pso.tile([128, 1024], f32, tag="o", name=f"po{tt}")
                            for p in range(NP):
                                for nb in range(2):
                                    nc.tensor.matmul(
                                        po[:, nb * 512 : (nb + 1) * 512],
                                        yt_sb[:, p, tt * 128 : (tt + 1) * 128],
                                        wp_sb[:, p, nb * 512 : (nb + 1) * 512],
                                        start=(p == 0), stop=(p == NP - 1),
                                    )
                            ot = outp2.tile([128, 1024], f32, tag="ot")
                            nc.scalar.copy(ot[:], po[:])
                            nc.sync.dma_start(
                                out_d[tt * 128 : (tt + 1) * 128, :], ot[:],
                            )
                wqk_cm.__exit__(None, None, None)
                xtp_cm.__exit__(None, None, None)
                vp_cm.__exit__(None, None, None)

            if rep == 1:
                body()
            else:
                with tc.For_i(0, rep, 1):
                    body()

    nc.compile()
    return nc


@functools.lru_cache(maxsize=None)
def _get_nc(rep=1, la=2, sbufs=3, ybufs=2, ptbufs=6, ablate="full"):
    return _build(rep, la, sbufs, ybufs, ptbufs, ablate)


def make_in_maps(x, w_qkv, w_proj):
    import ml_dtypes
    bf16 = ml_dtypes.bfloat16
    j = np.arange(128)[None, :]
    i = np.arange(128)[:, None]
    tri = (j >= i).astype(bf16)
    in_maps = []
    for core in range(N_CORES):
        b, hg = divmod(core, HG)
        sl = slice(hg * 512, (hg + 1) * 512)
        in_maps.append({
            "xt": np.ascontiguousarray(x[b].T).astype(bf16),
            "wq": np.ascontiguousarray(w_qkv[sl].T).astype(bf16),
            "wk": np.ascontiguousarray(w_qkv[C:2 * C][sl].T).astype(bf16),
            "wv": np.ascontiguousarray(w_qkv[2 * C:3 * C][sl].T).astype(bf16),
            "wp": np.ascontiguousarray(w_proj[:, sl].T).astype(bf16),
            "tri": tri,
        })
    return in_maps


def combine(results):
    out = np.empty((B, T, C), dtype=np.float32)
    for b in range(B):
        out[b] = results[2 * b]["out"] + results[2 * b + 1]["out"]
    return out


# ---------------------------------------------------------------------------
# PJRT runner (device-resident inputs, reusable jitted executable)
# ---------------------------------------------------------------------------

class _Runner:
    def __init__(self, nc, n_cores=N_CORES):
        import jax
        import concourse.mybir as mybir
        from concourse import bass2jax
        from jax.sharding import Mesh, PartitionSpec, NamedSharding
        from jax.experimental.shard_map import shard_map

        self.jax = jax
        bass2jax.install_neuronx_cc_hook()
        partition_name = (
            nc.partition_id_tensor.name if nc.partition_id_tensor else None
        )
        in_names, out_names, out_avals, zero_outs = [], [], [], []
        for alloc in nc.m.functions[0].allocations:
            if not isinstance(alloc, mybir.MemoryLocationSet):
                continue
            name = alloc.memorylocations[0].name
            if alloc.kind == "ExternalInput":
                if name != partition_name:
                    in_names.append(name)
            elif alloc.kind == "ExternalOutput":
                out_names.append(name)
                shape = tuple(alloc.tensor_shape)
                dtype = mybir.dt.np(alloc.dtype)
                out_avals.append(jax.core.ShapedArray(shape, dtype))
                zero_outs.append(np.zeros(shape, dtype))
        self.in_names, self.out_names = in_names, out_names
        self.out_avals, self.zero_outs = out_avals, zero_outs
        self.n_cores = n_cores
        all_names = in_names + out_names
        if partition_name is not None:
            all_names = all_names + [partition_name]

        def _bdy(*args):
            operands = list(args)
            if partition_name is not None:
                operands.append(bass2jax.partition_id_tensor())
            outs = bass2jax._bass_exec_p.bind(
                *operands,
                out_avals=tuple(out_avals),
                in_names=tuple(all_names),
                out_names=tuple(out_names),
                lowering_input_output_aliases=(),
                sim_require_finite=True,
                sim_require_nnan=True,
                nc=nc,
            )
            return tuple(outs)

        devices = jax.devices()[:n_cores]
        mesh = Mesh(np.asarray(devices), ("core",))
        n_args = len(in_names) + len(out_names)
        self.fn = jax.jit(
            shard_map(
                _bdy, mesh=mesh,
                in_specs=(PartitionSpec("core"),) * n_args,
                out_specs=(PartitionSpec("core"),) * len(out_names),
                check_rep=False,
            ),
            keep_unused=True,
        )
        self.sharding = NamedSharding(mesh, PartitionSpec("core"))

    def put_inputs(self, in_maps):
        concat = [
            np.concatenate([np.asarray(m[name]) for m in in_maps], axis=0)
            for name in self.in_names
        ]
        concat += [
            np.zeros((self.n_cores * z.shape[0], *z.shape[1:]), z.dtype)
            for z in self.zero_outs
        ]
        self.args = [self.jax.device_put(a, self.sharding) for a in concat]
        self.jax.block_until_ready(self.args)

    def run(self):
        outs = self.fn(*self.args)
        self.jax.block_until_ready(outs)
        return [
            {
                name: np.asarray(outs[i]).reshape(
                    self.n_cores, *self.out_avals[i].shape)[c]
                for i, name in enumerate(self.out_names)
            }
            for c in range(self.n_cores)
        ]

    def time_ns(self, iters=20, warmup=2):
        import time
        for _ in range(warmup):
            self.jax.block_until_ready(self.fn(*self.args))
        t0 = time.perf_counter()
        outs = None
        for _ in range(iters):
            outs = self.fn(*self.args)
        self.jax.block_until_ready(outs)
        t1 = time.perf_counter()
        return (t1 - t0) / iters * 1e9



@functools.lru_cache(maxsize=None)
def _get_runner(rep=1, la=2, sbufs=3, ybufs=2, ptbufs=6, ablate="full"):
    return _Runner(_get_nc(rep, la, sbufs, ybufs, ptbufs, ablate))


def kernel(x, w_qkv, w_proj):
    x = np.asarray(x, dtype=np.float32)
    w_qkv = np.asarray(w_qkv, dtype=np.float32)
    w_proj = np.asarray(w_proj, dtype=np.float32)
    runner = _get_runner()
    runner.put_inputs(make_in_maps(x, w_qkv, w_proj))
    return combine(runner.run())



# revision 8
# speedup vs baseline: 1.3642x; 1.3642x over previous
"""Causal self-attention v3 (B=4, T=2048, C=1024, H=16) on 8 trn2 NeuronCores.

Sharding: core = (batch b, head-group hg), b = core//2, hg = core%2 (Megatron
column-parallel qkv / row-parallel proj); host sums the two partial outputs.

v3 changes vs v2:
  - qb-major schedule: proj tasks (a=QK, b=V, d=out-proj) woven INTO the
    attention group stream so the PE never idles while ACT (exp) works;
    d(qb-1) runs during C(qb).
  - normalization: DVE reciprocal straight off the PSUM rowsum row, GPSIMD
    partition_broadcast (idle engine) for the [64,512] broadcast, single DVE
    mul per head.  No more PE broadcast matmuls / rs / recb / bc_sb hops.
  - S/pt tiles are [128, 2, 512] so straddle groups get ONE exp over a
    strided AP instead of two (saves 352 ACT cycles per instruction).
  - proj PSUM tiles are one bank each ([128,512]) in their own pool, so an
    in-flight a/b/d task no longer blocks the S-tile pipeline.
  - PSUM budget: s 2x2 banks + y 2x1 + proj 2x1 = 8 banks.
"""

import functools

import numpy as np

B, T, C, H = 4, 2048, 1024, 16
HD = C // H  # 64
N_CORES = 8
HG = 2  # head groups
NH = H // HG  # heads per core = 8
NP = NH // 2  # head pairs per core = 4
TT = T // 128  # 16 t-tiles
TB = T // 512  # 4 t-blocks
CK = C // 128  # 8 c-chunks


def _build(rep=1, la=3, sbufs=2, ybufs=2, pbufs=2, ptbufs=6, ablate="full",
           norm="sbuf"):
    import concourse.bass as bass
    import concourse.mybir as mybir
    import concourse.tile as tile
    from concourse import bacc

    f32 = mybir.dt.float32
    bf16 = mybir.dt.bfloat16

    nc = bacc.Bacc("TRN2", target_bir_lowering=False, debug=False)

    xt_d = nc.dram_tensor("xt", [C, T], bf16, kind="ExternalInput")
    wq_d = nc.dram_tensor("wq", [C, 512], bf16, kind="ExternalInput")
    wk_d = nc.dram_tensor("wk", [C, 512], bf16, kind="ExternalInput")
    wv_d = nc.dram_tensor("wv", [C, 512], bf16, kind="ExternalInput")
    wp_d = nc.dram_tensor("wp", [512, C], bf16, kind="ExternalInput")
    tri_d = nc.dram_tensor("tri", [128, 128], bf16, kind="ExternalInput")
    out_d = nc.dram_tensor("out", [T, C], f32, kind="ExternalOutput")

    do_attn = ablate in ("full", "noav")
    do_av = ablate in ("full",)
    do_d = ablate in ("full", "noattn")

    with tile.TileContext(nc) as tc:
        with tc.tile_pool(name="persist", bufs=1) as persist:
            qt_sb = persist.tile([128, NP, T], bf16, tag="qt")
            kt_sb = persist.tile([128, NP, T], bf16, tag="kt")

            def body():
                # strictly nested (LIFO) pool lifetimes
                vp_cm = tc.tile_pool(name="vp", bufs=1)
                xtp_cm = tc.tile_pool(name="xtp", bufs=1)
                wqk_cm = tc.tile_pool(name="wqk", bufs=1)
                vp = vp_cm.__enter__()
                xtp = xtp_cm.__enter__()
                wqk = wqk_cm.__enter__()

                v_sb = vp.tile([128, TT, NH, HD + 1], bf16, tag="v")
                # ones columns of V': memset whole tile; V writes overlay 0:HD
                nc.vector.memset(v_sb[:], 1.0)

                wq_sb = wqk.tile([128, CK, 512], bf16, tag="wq")
                wk_sb = wqk.tile([128, CK, 512], bf16, tag="wk")
                wv_sb = wqk.tile([128, CK, 512], bf16, tag="wv")
                wp_sb = wqk.tile([128, NP, C], bf16, tag="wp")
                tri_sb = wqk.tile([128, 128], bf16, tag="tri")
                xt_sb = xtp.tile([128, CK, T], bf16, tag="xt")
                # exp-table preload on the idle ACT engine during the DMA head
                warm = wqk.tile([128, 32], bf16, tag="warm")
                nc.vector.memset(warm[:], 1.0)
                nc.scalar.activation(
                    warm[0:1, 16:32], warm[0:1, 0:16],
                    mybir.ActivationFunctionType.Exp, scale=0.125,
                )
                # chunked DMAs in consumption order; tri first (first-unit
                # masks); xt for tb=0 first so phase A starts early.
                nc.sync.dma_start(tri_sb[:], tri_d[:, :])
                for k in range(CK):
                    ksl = slice(k * 128, (k + 1) * 128)
                    nc.sync.dma_start(wq_sb[:, k, :], wq_d[ksl, :])
                    nc.sync.dma_start(wk_sb[:, k, :], wk_d[ksl, :])
                    nc.sync.dma_start(xt_sb[:, k, 0:512], xt_d[ksl, 0:512])
                    nc.sync.dma_start(wv_sb[:, k, :], wv_d[ksl, :])
                for tb in range(1, TB):
                    tsl = slice(tb * 512, (tb + 1) * 512)
                    for k in range(CK):
                        ksl = slice(k * 128, (k + 1) * 128)
                        nc.sync.dma_start(xt_sb[:, k, tsl], xt_d[ksl, tsl])
                nc.sync.dma_start(wp_sb[:], wp_d.rearrange("(a p) n -> p a n", p=128))

                with (
                    tc.tile_pool(name="persist2", bufs=1) as persist2,
                    tc.tile_pool(name="ptp", bufs=ptbufs) as ptp,
                    tc.tile_pool(name="recp", bufs=4) as recp,
                    tc.tile_pool(name="bcp", bufs=4) as bcp,
                    tc.tile_pool(name="outp", bufs=3) as outp,
                ):
                    yt_sb = persist2.tile([128, NP, T], bf16, tag="yt")
                    if ablate in ("noattn",):
                        nc.vector.memset(yt_sb[:], 0.001)
                    with (
                        tc.tile_pool(name="pss", bufs=sbufs, space="PSUM") as pss,
                        tc.tile_pool(name="psy", bufs=ybufs, space="PSUM") as psy,
                        tc.tile_pool(name="psp", bufs=pbufs, space="PSUM") as psp,
                    ):
                        # ---- task list: qb-major, proj tasks woven in
                        tasks = []
                        if do_attn:
                            for pp in range(NP):
                                tasks.append(("a", pp, 0, 0, 0))
                            for tt in range(4):
                                tasks.append(("b", tt, 0, 0, 0))
                            for qb in range(TB):
                                inter = []
                                if qb < TB - 1:
                                    for pp in range(NP):
                                        inter.append(("a", pp, qb + 1, 0, 0))
                                    for tt in range(4 * qb + 4, 4 * qb + 8):
                                        inter.append(("b", tt, 0, 0, 0))
                                if do_d and qb > 0:
                                    for tt in range(4 * (qb - 1), 4 * qb):
                                        inter.append(("d", tt, 0, 0, 0))
                                ng = 4 * (qb + 1)
                                glist = [
                                    ("g", qb, p, g, ng)
                                    for p in range(NP)
                                    for g in range(ng)
                                ]
                                if inter:
                                    step = max(1, len(glist) // len(inter))
                                    woven, ii = [], 0
                                    for j, t in enumerate(glist):
                                        woven.append(t)
                                        if j % step == step - 1 and ii < len(inter):
                                            woven.append(inter[ii])
                                            ii += 1
                                    woven += inter[ii:]
                                    glist = woven
                                tasks += glist
                            if do_d:
                                for tt in range(TT - 4, TT):
                                    tasks.append(("d", tt, 0, 0, 0))
                        elif do_d:
                            for tt in range(TT):
                                tasks.append(("d", tt, 0, 0, 0))

                        pt_store = {}
                        s_store = {}
                        y_store = {}

                        def emit_front(idx):
                            kind, qb, p, g, ng = tasks[idx]
                            if kind != "g":
                                return
                            r = g - 4 * qb  # >=0: diagonal-straddling chunk
                            lo = 128 * r if r > 0 else 0
                            ksl = slice(g * 128, (g + 1) * 128)
                            s = pss.tile([128, 2, 512], f32, tag="s",
                                         name=f"s_{idx}")
                            for hf in range(2):
                                nc.tensor.matmul(
                                    s[:, hf, lo:512],
                                    kt_sb[64 * hf: 64 * (hf + 1), p, ksl],
                                    qt_sb[64 * hf: 64 * (hf + 1), p,
                                          qb * 512 + lo: (qb + 1) * 512],
                                    start=True, stop=True,
                                )
                            pt = ptp.tile([128, 2, 512], bf16, tag="pt",
                                          name=f"pt_{idx}")
                            # one exp per group (strided AP when straddling)
                            nc.scalar.activation(
                                pt[:, :, lo:512], s[:, :, lo:512],
                                mybir.ActivationFunctionType.Exp, scale=0.125,
                            )
                            if r >= 0:
                                # triangle mask on the diagonal 128-wide strip
                                for hf in range(2):
                                    nc.vector.tensor_mul(
                                        pt[:, hf, lo:lo + 128],
                                        pt[:, hf, lo:lo + 128], tri_sb[:])
                            pt_store[idx] = pt

                        def emit_back(idx):
                            kind, qb, p, g, ng = tasks[idx]
                            if kind == "a":
                                pp, tb = qb, p
                                tsl = slice(tb * 512, (tb + 1) * 512)
                                psl = slice(pp * 128, (pp + 1) * 128)
                                psq = psp.tile([128, 512], f32, tag="o",
                                               name=f"psq{pp}_{tb}")
                                for k in range(CK):
                                    nc.tensor.matmul(
                                        psq[:], wq_sb[:, k, psl],
                                        xt_sb[:, k, tsl],
                                        start=(k == 0), stop=(k == CK - 1),
                                    )
                                nc.vector.tensor_copy(qt_sb[:, pp, tsl], psq[:])
                                psk = psp.tile([128, 512], f32, tag="o",
                                               name=f"psk{pp}_{tb}")
                                for k in range(CK):
                                    nc.tensor.matmul(
                                        psk[:], wk_sb[:, k, psl],
                                        xt_sb[:, k, tsl],
                                        start=(k == 0), stop=(k == CK - 1),
                                    )
                                nc.vector.tensor_copy(kt_sb[:, pp, tsl], psk[:])
                                return
                            if kind == "b":
                                tt = qb
                                psv = psp.tile([128, 512], f32, tag="o",
                                               name=f"psv{tt}")
                                for k in range(CK):
                                    nc.tensor.matmul(
                                        psv[:],
                                        xt_sb[:, k, tt * 128: (tt + 1) * 128],
                                        wv_sb[:, k, :],
                                        start=(k == 0), stop=(k == CK - 1),
                                    )
                                nc.vector.tensor_copy(
                                    v_sb[:, tt, :, 0:HD],
                                    psv[:].rearrange("p (h e) -> p h e", e=HD),
                                )
                                return
                            if kind == "d":
                                tt = qb
                                tsl = slice(tt * 128, (tt + 1) * 128)
                                for nb in range(2):
                                    po = psp.tile([128, 512], f32, tag="o",
                                                  name=f"po{tt}_{nb}")
                                    for pp2 in range(NP):
                                        nc.tensor.matmul(
                                            po[:],
                                            yt_sb[:, pp2, tsl],
                                            wp_sb[:, pp2,
                                                  nb * 512: (nb + 1) * 512],
                                            start=(pp2 == 0),
                                            stop=(pp2 == NP - 1),
                                        )
                                    ot = outp.tile([128, 512], f32, tag="ot")
                                    nc.vector.tensor_copy(ot[:], po[:])
                                    nc.sync.dma_start(
                                        out_d[tsl, nb * 512: (nb + 1) * 512],
                                        ot[:],
                                    )
                                return
                            # kind == "g"
                            r = g - 4 * qb
                            lo = 128 * r if r > 0 else 0
                            pt = pt_store.pop(idx)
                            if (qb, p) not in y_store:
                                ya = psy.tile([65, 512], f32, tag="y",
                                              name=f"ya_{qb}_{p}")
                                yb = psy.tile([65, 512], f32, tag="y",
                                              name=f"yb_{qb}_{p}")
                                y_store[(qb, p)] = (ya, yb)
                            ya, yb = y_store[(qb, p)]
                            if do_av:
                                for hf, yy in ((0, ya), (1, yb)):
                                    nc.tensor.matmul(
                                        yy[:, lo:512],
                                        v_sb[:, g, 2 * p + hf, :],
                                        pt[:, hf, lo:512],
                                        start=(g == 0), stop=(g == ng - 1),
                                        skip_group_check=True,
                                    )
                            if g != ng - 1:
                                return
                            if not do_av:
                                nc.vector.memset(ya[:], 1.0)
                                nc.vector.memset(yb[:], 1.0)
                            # normalize: yt = y[0:64] * (1 / rowsum)
                            qsl = slice(qb * 512, (qb + 1) * 512)
                            for hi, yy in ((0, ya), (1, yb)):
                                rec = recp.tile([1, 512], f32, tag="rec",
                                                name=f"rec_{qb}_{p}_{hi}")
                                if norm == "gps":
                                    nc.vector.reciprocal_approx_fast(
                                        rec[0:1, :], yy[64:65, :])
                                else:  # "sbuf": stage rowsum in SBUF first
                                    rs = recp.tile([1, 512], f32, tag="rs",
                                                   name=f"rs_{qb}_{p}_{hi}")
                                    nc.vector.tensor_copy(
                                        rs[0:1, :], yy[64:65, :])
                                    nc.vector.reciprocal_approx_fast(
                                        rec[0:1, :], rs[0:1, :])
                                bc = bcp.tile([64, 512], f32, tag="bc",
                                              name=f"bc_{qb}_{p}_{hi}")
                                nc.gpsimd.partition_broadcast(
                                    bc[:], rec[0:1, :], channels=64)
                                nc.vector.tensor_mul(
                                    yt_sb[hi * 64: (hi + 1) * 64, p, qsl],
                                    yy[0:64, :], bc[:],
                                )
                            del y_store[(qb, p)]

                        n = len(tasks)
                        for j in range(min(la, n)):
                            emit_front(j)
                        for i in range(n):
                            if i + la < n:
                                emit_front(i + la)
                            emit_back(i)

                wqk_cm.__exit__(None, None, None)
                xtp_cm.__exit__(None, None, None)
                vp_cm.__exit__(None, None, None)

            if rep == 1:
                body()
            else:
                with tc.For_i(0, rep, 1):
                    body()

    nc.compile()
    return nc


@functools.lru_cache(maxsize=None)
def _get_nc(rep=1, la=3, sbufs=2, ybufs=2, pbufs=2, ptbufs=6, ablate="full",
            norm="sbuf"):
    return _build(rep, la, sbufs, ybufs, pbufs, ptbufs, ablate, norm)


def make_in_maps(x, w_qkv, w_proj):
    import ml_dtypes
    bf16 = ml_dtypes.bfloat16
    j = np.arange(128)[None, :]
    i = np.arange(128)[:, None]
    tri = (j >= i).astype(bf16)
    in_maps = []
    for core in range(N_CORES):
        b, hg = divmod(core, HG)
        sl = slice(hg * 512, (hg + 1) * 512)
        in_maps.append({
            "xt": np.ascontiguousarray(x[b].T).astype(bf16),
            "wq": np.ascontiguousarray(w_qkv[sl].T).astype(bf16),
            "wk": np.ascontiguousarray(w_qkv[C:2 * C][sl].T).astype(bf16),
            "wv": np.ascontiguousarray(w_qkv[2 * C:3 * C][sl].T).astype(bf16),
            "wp": np.ascontiguousarray(w_proj[:, sl].T).astype(bf16),
            "tri": tri,
        })
    return in_maps


def combine(results):
    out = np.empty((B, T, C), dtype=np.float32)
    for b in range(B):
        out[b] = results[2 * b]["out"] + results[2 * b + 1]["out"]
    return out


# ---------------------------------------------------------------------------
# PJRT runner (device-resident inputs, reusable jitted executable)
# ---------------------------------------------------------------------------

class _Runner:
    def __init__(self, nc, n_cores=N_CORES):
        import jax
        import concourse.mybir as mybir
        from concourse import bass2jax
        from jax.sharding import Mesh, PartitionSpec, NamedSharding
        from jax.experimental.shard_map import shard_map

        self.jax = jax
        bass2jax.install_neuronx_cc_hook()
        partition_name = (
            nc.partition_id_tensor.name if nc.partition_id_tensor else None
        )
        in_names, out_names, out_avals, zero_outs = [], [], [], []
        for alloc in nc.m.functions[0].allocations:
            if not isinstance(alloc, mybir.MemoryLocationSet):
                continue
            name = alloc.memorylocations[0].name
            if alloc.kind == "ExternalInput":
                if name != partition_name:
                    in_names.append(name)
            elif alloc.kind == "ExternalOutput":
                out_names.append(name)
                shape = tuple(alloc.tensor_shape)
                dtype = mybir.dt.np(alloc.dtype)
                out_avals.append(jax.core.ShapedArray(shape, dtype))
                zero_outs.append(np.zeros(shape, dtype))
        self.in_names, self.out_names = in_names, out_names
        self.out_avals, self.zero_outs = out_avals, zero_outs
        self.n_cores = n_cores
        all_names = in_names + out_names
        if partition_name is not None:
            all_names = all_names + [partition_name]

        def _bdy(*args):
            operands = list(args)
            if partition_name is not None:
                operands.append(bass2jax.partition_id_tensor())
            outs = bass2jax._bass_exec_p.bind(
                *operands,
                out_avals=tuple(out_avals),
                in_names=tuple(all_names),
                out_names=tuple(out_names),
                lowering_input_output_aliases=(),
                sim_require_finite=True,
                sim_require_nnan=True,
                nc=nc,
            )
            return tuple(outs)

        devices = jax.devices()[:n_cores]
        mesh = Mesh(np.asarray(devices), ("core",))
        n_args = len(in_names) + len(out_names)
        self.fn = jax.jit(
            shard_map(
                _bdy, mesh=mesh,
                in_specs=(PartitionSpec("core"),) * n_args,
                out_specs=(PartitionSpec("core"),) * len(out_names),
                check_rep=False,
            ),
            keep_unused=True,
        )
        self.sharding = NamedSharding(mesh, PartitionSpec("core"))

    def put_inputs(self, in_maps):
        concat = [
            np.concatenate([np.asarray(m[name]) for m in in_maps], axis=0)
            for name in self.in_names
        ]
        concat += [
            np.zeros((self.n_cores * z.shape[0], *z.shape[1:]), z.dtype)
            for z in self.zero_outs
        ]
        self.args = [self.jax.device_put(a, self.sharding) for a in concat]
        self.jax.block_until_ready(self.args)

    def run(self):
        outs = self.fn(*self.args)
        self.jax.block_until_ready(outs)
        return [
            {
                name: np.asarray(outs[i]).reshape(
                    self.n_cores, *self.out_avals[i].shape)[c]
                for i, name in enumerate(self.out_names)
            }
            for c in range(self.n_cores)
        ]

    def time_ns(self, iters=20, warmup=2):
        import time
        for _ in range(warmup):
            self.jax.block_until_ready(self.fn(*self.args))
        t0 = time.perf_counter()
        outs = None
        for _ in range(iters):
            outs = self.fn(*self.args)
        self.jax.block_until_ready(outs)
        t1 = time.perf_counter()
        return (t1 - t0) / iters * 1e9



@functools.lru_cache(maxsize=None)
def _get_runner(rep=1, la=3, sbufs=2, ybufs=2, pbufs=2, ptbufs=6, ablate="full",
                norm="sbuf"):
    return _Runner(_get_nc(rep, la, sbufs, ybufs, pbufs, ptbufs, ablate, norm))


def kernel(x, w_qkv, w_proj):
    x = np.asarray(x, dtype=np.float32)
    w_qkv = np.asarray(w_qkv, dtype=np.float32)
    w_proj = np.asarray(w_proj, dtype=np.float32)
    runner = _get_runner()
    runner.put_inputs(make_in_maps(x, w_qkv, w_proj))
    return combine(runner.run())
